# revision 1
# baseline (speedup 1.0000x reference)
"""Trainium2 Bass kernel for BertWithAdaThresholdLocContextPooling head.

Data-parallel over batch: 32 batches -> 8 NeuronCores x 4 batches.
Per core, everything is gather-based: only 8 mention rows of
sequence_output (entity 0) and 16*12 attention rows per batch are read
via indirect DMA -- the 400MB attention tensor is never fully read.

Math per batch b (faithful to the reference, including the hs-in-both-
extractors detail):
  hs  = logsumexp_m seq[pos[b,0,m]]                       [768]
  A_e = mean_m attention[:, pos[b,e,m], :]                [12, 512]
  w   = sum_h A_0 * A_1;  rs = (w @ seq[b]) / (sum(w) + 12e-5)
  x_f = tanh(W_f @ [hs | rs | ner_f | 1])   f in {head, tail}
  logits = W_bil @ vec(outer-per-group(x_head, x_tail)) + b_bil

All matmuls run with the contraction dim on SBUF partitions; activations
are kept feature-on-partition ("transposed") end to end so no on-device
transposes are needed. The grouped outer product is built with constant
32x128 replication matmuls. Weights/activations are bf16 (f32 PSUM
accumulation); the attention path stays f32 until the head product.
"""

import os

import numpy as np

import concourse.bass as bass
import concourse.tile as tile
from concourse import bacc, mybir
from concourse.bass_utils import run_bass_kernel_spmd

# problem dims
B, H, C, D = 32, 12, 512, 768
M = 8
EMB, BLK = 768, 8
NCLS, NER = 97, 6
OFFSET = 1
NCORES = 8
BL = B // NCORES            # batches per core
KIN = 2 * D + NER + 1       # 1543: [hs | rs | ner | 1]
KCH = 13                    # ceil(KIN / 128)
KLAST = KIN - 12 * 128      # 7 rows in the last chunk
KP = EMB * BLK              # 6144
NT = KP // 128              # 48 bilinear chunks
F32 = mybir.dt.float32
BF16 = mybir.dt.bfloat16
I32 = mybir.dt.int32

# f32 const block [128, _CF_NCOL]
_CF_SEL96 = 0       # [96,12]  mention-mean selector (1/M)
_CF_ONES128 = 12    # [1,128]
_CF_BBIL = 140      # [97,1]   bilinear bias
_CF_ONESC = 141     # [128,1]
_CF_NCOL = 142
# bf16 const block [128, _CB_NCOL]
_CB_SEL32 = 0       # [32,4]   mention->batch sum selector
_CB_ONES12 = 4      # [12,1]
_CB_NERH = 5        # [7,4]    [ner0 | 1] per batch (extractor last chunk)
_CB_NERT = 9        # [7,4]    [ner1 | 1]
_CB_SAB = 13        # 8 x [64,128] bilinear row replicators, tiled at bases
_CB_NCOL = 13 + 8 * 128   # 0/64; cols = [A_v0..A_v3 | B_v0..B_v3]

_CACHE = {}

LAST_EXEC_NS = None
LAST_RESULTS = None


def _build_nc():
    nc = bacc.Bacc("TRN2", target_bir_lowering=False, debug=False)

    seq_h = nc.dram_tensor("seq", [BL * C, D], BF16, kind="ExternalInput")
    attn_h = nc.dram_tensor("attn", [BL * H * C, C], F32, kind="ExternalInput")
    idx_h = nc.dram_tensor("idx", [M * H, 1 + 2 * BL], I32, kind="ExternalInput")
    wh_h = nc.dram_tensor("whT", [128, KCH * EMB], BF16, kind="ExternalInput")
    wt_h = nc.dram_tensor("wtT", [128, KCH * EMB], BF16, kind="ExternalInput")
    wb_h = nc.dram_tensor("wbT", [128, NT * NCLS], BF16, kind="ExternalInput")
    cstf_h = nc.dram_tensor("cstf", [128, _CF_NCOL], F32, kind="ExternalInput")
    cstb_h = nc.dram_tensor("cstb", [128, _CB_NCOL], BF16, kind="ExternalInput")
    out_h = nc.dram_tensor("outT", [NCLS, BL], F32, kind="ExternalOutput")

    AF = mybir.ActivationFunctionType
    OP = mybir.AluOpType

    with tile.TileContext(nc) as tc:
        with (
            tc.tile_pool(name="w", bufs=1) as wp,
            tc.tile_pool(name="seqp", bufs=1) as sp,
            tc.tile_pool(name="attp", bufs=1) as ap,
            tc.tile_pool(name="g", bufs=2) as gp,
            tc.tile_pool(name="blp", bufs=3) as blp,
            tc.tile_pool(name="ps", bufs=8, space="PSUM") as pp,
        ):
            # --- consolidated small loads first on the ACT queue
            idx_sb = wp.tile([M * H, 1 + 2 * BL], I32)
            nc.sync.dma_start(out=idx_sb[:], in_=idx_h[:])
            cstf_sb = wp.tile([128, _CF_NCOL], F32)
            nc.sync.dma_start(out=cstf_sb[:], in_=cstf_h[:])
            cstb_sb = wp.tile([128, _CB_NCOL], BF16)
            nc.sync.dma_start(out=cstb_sb[:], in_=cstb_h[:])

            sel96 = cstf_sb[0 : M * H, _CF_SEL96 : _CF_SEL96 + H]
            ones128 = cstf_sb[0:1, _CF_ONES128 : _CF_ONES128 + 128]
            bbil = cstf_sb[0:NCLS, _CF_BBIL : _CF_BBIL + 1]
            onescol = cstf_sb[0:128, _CF_ONESC : _CF_ONESC + 1]
            sel32 = cstb_sb[0 : BL * M, _CB_SEL32 : _CB_SEL32 + BL]
            ones12 = cstb_sb[0:H, _CB_ONES12 : _CB_ONES12 + 1]
            nerh = cstb_sb[0:KLAST, _CB_NERH : _CB_NERH + BL]
            nert = cstb_sb[0:KLAST, _CB_NERT : _CB_NERT + BL]

            # batch-3 seq tiles early on SP (ACT's tail seq would gate rs)
            seq_sp = {}
            for i in range(12, 16):
                seq_t = sp.tile([128, D], BF16, name=f"seq_t{i}")
                nc.sync.dma_start(out=seq_t[:], in_=seq_h[i * 128 : (i + 1) * 128, :])
                seq_sp[i] = seq_t

            # inpT[:, c, :] = chunk c of [hs | rs] with features on partitions
            inpT = wp.tile([128, 12, BL], BF16)

            # ---- phase 1: mention gather + logsumexp -> inpT chunks 0..5
            g_ment = gp.tile([BL * M, D], BF16)
            nc.gpsimd.indirect_dma_start(
                out=g_ment[:],
                out_offset=None,
                in_=seq_h[:],
                in_offset=bass.IndirectOffsetOnAxis(ap=idx_sb[0 : BL * M, 0:1], axis=0),
            )
            # all 8 attention-row gathers up front on the Pool queue,
            # into resident tiles (no slot recycling stalls)
            att_gs = {}
            for b in range(BL):
                for e in range(2):
                    col = 1 + b * 2 + e
                    att_g = ap.tile([M * H, C], F32, name=f"att_g{b}_{e}")
                    last_gather = nc.gpsimd.indirect_dma_start(
                        out=att_g[:],
                        out_offset=None,
                        in_=attn_h[:],
                        in_offset=bass.IndirectOffsetOnAxis(
                            ap=idx_sb[:, col : col + 1], axis=0
                        ),
                    )
                    att_gs[b, e] = att_g

            seq_ts = []
            for i in range(8):
                seq_t = sp.tile([128, D], BF16, name=f"seq_t{i}")
                nc.scalar.dma_start(out=seq_t[:], in_=seq_h[i * 128 : (i + 1) * 128, :])
                seq_ts.append(seq_t)

            expm = gp.tile([BL * M, D], BF16)
            nc.scalar.activation(expm[:], g_ment[:], AF.Exp)
            # all 6 chunk sums share one single-bank PSUM tile
            lse_ps = pp.tile([128, 6 * BL], F32, tag="ps", name="lse_ps")
            for c in range(6):
                nc.tensor.matmul(
                    out=lse_ps[:, c * BL : (c + 1) * BL],
                    lhsT=expm[:, c * 128 : (c + 1) * 128],
                    rhs=sel32,
                    start=True,
                    stop=True,
                )
            nc.scalar.activation(inpT[:, 0:6, :], lse_ps[:], AF.Ln)

            # --- remaining seq on ACT; whT on SP; wbT behind the gathers on
            # Pool; wtT split in thirds across all three DMA queues
            for i in range(8, 12):
                seq_t = sp.tile([128, D], BF16, name=f"seq_t{i}")
                nc.scalar.dma_start(out=seq_t[:], in_=seq_h[i * 128 : (i + 1) * 128, :])
                seq_ts.append(seq_t)
            seq_ts.extend(seq_sp[i] for i in range(12, 16))
            wh_sb = wp.tile([128, KCH * EMB], BF16)
            nc.sync.dma_start(out=wh_sb[:], in_=wh_h[:])
            # wtT pieces sized so all three queues finish it ~together;
            # on Pool it goes ahead of wbT (needed earlier)
            wt_sb = wp.tile([128, KCH * EMB], BF16)
            nc.gpsimd.dma_start(out=wt_sb[:, 5888:], in_=wt_h[:, 5888:])
            nc.sync.dma_start(out=wt_sb[:, 0:2176], in_=wt_h[:, 0:2176])
            nc.scalar.dma_start(out=wt_sb[:, 2176:5888], in_=wt_h[:, 2176:5888])
            wb_sb = wp.tile([128, NT * NCLS], BF16)
            nc.gpsimd.dma_start(out=wb_sb[:], in_=wb_h[:])

            # ---- phase 2: attention means -> normalized context weights
            # wTn_sb[:, 4*b + c] = ht_att[b, c*128 + p] (c = seq chunk)
            wTn_sb = wp.tile([128, BL * 4], BF16)
            sraw = wp.tile([1, BL], F32)
            sden = wp.tile([1, BL], F32)
            srec = wp.tile([1, BL], F32)
            for b in range(BL):
                # transposed mention-mean: PT[c*128+p, e, h] then
                # w^T[:, c] = sum_h PT0*PT1 fused on DVE
                wT_f = blp.tile([128, 4], F32, tag="wT_f")
                PT = pp.tile([128, 4, 2, H], F32, tag="ps", name="PT")
                for c in range(4):
                    for e in range(2):
                        nc.tensor.matmul(
                            out=PT[:, c, e, :],
                            lhsT=att_gs[b, e][:, c * 128 : (c + 1) * 128],
                            rhs=sel96, start=True, stop=True,
                        )
                # DVE may read only one PSUM operand: stage the e=0 half in SBUF
                pte0 = blp.tile([128, 4, H], F32, tag="pte0")
                nc.vector.tensor_copy(pte0[:, :, :], PT[:, :, 0, :])
                prodT = blp.tile([128, 4, H], F32, tag="prodT")
                nc.vector.tensor_tensor(
                    out=prodT[:, :, :], in0=pte0[:, :, :], in1=PT[:, :, 1, :],
                    op=OP.mult,
                )
                nc.vector.reduce_sum(
                    out=wT_f[:], in_=prodT[:, :, :], axis=mybir.AxisListType.X
                )
                # s_b = sum(w) via PE (column sums then a 4-wide reduce)
                s_ps = pp.tile([1, 4], F32, tag="ps", name="s_ps")
                nc.tensor.matmul(
                    out=s_ps[:], lhsT=onescol, rhs=wT_f[:], start=True, stop=True
                )
                nc.vector.reduce_sum(
                    out=sraw[0:1, b : b + 1], in_=s_ps[:], axis=mybir.AxisListType.X
                )
                # ht = w / (sum(w) + H*1e-5); denominator folds the /H and /M^2
                nc.vector.tensor_scalar_add(
                    out=sden[0:1, b : b + 1],
                    in0=sraw[0:1, b : b + 1],
                    scalar1=float(H) * 1e-5,
                )
                nc.vector.reciprocal(out=srec[0:1, b : b + 1], in_=sden[0:1, b : b + 1])
                sb_ps = pp.tile([128, 1], F32, tag="ps", name="sb_ps")
                nc.tensor.matmul(
                    out=sb_ps[:],
                    lhsT=ones128,
                    rhs=srec[0:1, b : b + 1],
                    start=True,
                    stop=True,
                )
                nc.vector.tensor_tensor(
                    out=wTn_sb[:, b * 4 : (b + 1) * 4],
                    in0=wT_f[:],
                    in1=sb_ps[:, 0:1].to_broadcast([128, 4]),
                    op=OP.mult,
                )  # wTn_sb is bf16; DVE rounds on write

            # ---- phase 3: rs = ht_att @ seq  (resident seq tiles)
            # three banks, alternating groups so same-bank sem delays overlap;
            # each (b,j) group's 4 matmuls stay consecutive (lazy zero region)
            rsT = [
                pp.tile([128, 2, BL], F32, tag="ps", name=f"rsT{k}") for k in range(3)
            ]
            for b in range(BL):
                for j in range(6):
                    for c in range(4):
                        nc.tensor.matmul(
                            out=rsT[j % 3][:, j // 3, b : b + 1],
                            lhsT=seq_ts[b * 4 + c][:, j * 128 : (j + 1) * 128],
                            rhs=wTn_sb[:, (b * 4 + c) : (b * 4 + c) + 1],
                            start=(c == 0),
                            stop=(c == 3),
                        )
            for k in range(3):
                nc.vector.tensor_copy(inpT[:, 6 + k : 12 : 3, :], rsT[k][:, :, :])

            # ---- phase 4: extractors (features on partitions)
            # ex_fT[:, j, 0:4] = tanh head feats [j*128, (j+1)*128) for 4
            # batches; ex_fT[:, j, 4:8] = tanh tail feats
            ex_fT = wp.tile([128, 6, 2 * BL], BF16)
            corder = list(range(6, 12)) + [12] + list(range(6))  # rs first
            for j in range(6):
                # head + tail share one bank as two sequential groups
                ex_ps = pp.tile([128, 2 * BL], F32, tag="ps", name="ex_ps")
                for half, (w_sb, rner) in enumerate(((wh_sb, nerh), (wt_sb, nert))):
                    for ci, c in enumerate(corder):
                        if c < 12:
                            l = w_sb[:, c * EMB + j * 128 : c * EMB + (j + 1) * 128]
                            r = inpT[:, c, :]
                        else:
                            l = w_sb[0:KLAST, c * EMB + j * 128 : c * EMB + (j + 1) * 128]
                            r = rner
                        nc.tensor.matmul(
                            out=ex_ps[:, half * BL : (half + 1) * BL], lhsT=l, rhs=r,
                            start=(ci == 0), stop=(ci == KCH - 1),
                        )
                nc.scalar.activation(ex_fT[:, j, :], ex_ps[:], AF.Tanh)

            # ---- phase 5: grouped bilinear + output matmul
            logit_ps = pp.tile([NCLS, BL], F32, tag="ps", name="logit_ps")
            for tg in range(NT // 8):
                # expand only the needed half per side: head cols for A,
                # tail cols for B -- halves PSUM traffic and the DVE chain
                psA4 = pp.tile([128, 8 * BL], F32, tag="ps", name="psA4")
                psB4 = pp.tile([128, 8 * BL], F32, tag="ps", name="psB4")
                for i in range(8):
                    t = tg * 8 + i
                    j6, r = t // 8, t % 8
                    base, v = 64 * (r // 4), r % 4
                    srcH = ex_fT[base : base + 64, j6, 0:BL]
                    srcT = ex_fT[base : base + 64, j6, BL : 2 * BL]
                    selA = cstb_sb[base : base + 64,
                                   _CB_SAB + v * 128 : _CB_SAB + (v + 1) * 128]
                    selB = cstb_sb[base : base + 64,
                                   _CB_SAB + (4 + v) * 128 : _CB_SAB + (5 + v) * 128]
                    nc.tensor.matmul(
                        out=psA4[:, i * BL : (i + 1) * BL], lhsT=selA, rhs=srcH,
                        start=True, stop=True,
                    )
                    nc.tensor.matmul(
                        out=psB4[:, i * BL : (i + 1) * BL], lhsT=selB, rhs=srcT,
                        start=True, stop=True,
                    )
                # stage psA4 in SBUF (single-PSUM-operand rule)
                psA_sb = blp.tile([128, 8 * BL], F32, tag="psA_sb")
                nc.vector.tensor_copy(psA_sb[:], psA4[:])
                blT4 = blp.tile([128, 8, BL], BF16, tag="blT4")
                nc.vector.tensor_tensor(
                    out=blT4[:, :, :],
                    in0=psA_sb[:].rearrange("p (i c) -> p i c", c=BL),
                    in1=psB4[:].rearrange("p (i c) -> p i c", c=BL),
                    op=OP.mult,
                )
                for i in range(8):
                    t = tg * 8 + i
                    nc.tensor.matmul(
                        out=logit_ps[:],
                        lhsT=wb_sb[:, t * NCLS : (t + 1) * NCLS],
                        rhs=blT4[:, i, :],
                        start=(t == 0),
                        stop=(t == NT - 1),
                    )
            logitsT_sb = wp.tile([NCLS, BL], F32)
            nc.vector.tensor_scalar_add(out=logitsT_sb[:], in0=logit_ps[:], scalar1=bbil)
            nc.sync.dma_start(out=out_h[:], in_=logitsT_sb[:])

    nc.compile()
    return nc


def _bf16(x):
    import ml_dtypes

    return np.ascontiguousarray(np.asarray(x).astype(ml_dtypes.bfloat16))


def _weights_prep(W_head, b_head, W_tail, b_tail, W_bil, b_bil):
    """Host-side weight packing (transposed + chunk-interleaved + bias rows)."""

    def pack_ext(Wf, bf):
        ext = np.zeros((KCH * 128, EMB), np.float32)
        ext[: 2 * D + NER] = Wf.T.astype(np.float32)
        ext[2 * D + NER] = bf.astype(np.float32)
        return _bf16(
            ext.reshape(KCH, 128, EMB).transpose(1, 0, 2).reshape(128, KCH * EMB)
        )

    whT = pack_ext(W_head, b_head)
    wtT = pack_ext(W_tail, b_tail)

    wbe = np.asarray(W_bil, np.float32).T  # [KP, NCLS]
    wbT = _bf16(wbe.reshape(NT, 128, NCLS).transpose(1, 0, 2).reshape(128, NT * NCLS))

    cstf = np.zeros((128, _CF_NCOL), np.float32)
    for m in range(M):
        for h in range(H):
            cstf[m * H + h, _CF_SEL96 + h] = 1.0 / M
    cstf[0, _CF_ONES128 : _CF_ONES128 + 128] = 1.0
    cstf[0:NCLS, _CF_BBIL] = b_bil.astype(np.float32)
    cstf[0:128, _CF_ONESC] = 1.0
    return whT, wtT, wbT, cstf


def _cstb_prep(ner_slice):
    """Per-core bf16 const block: selectors + ner columns + bilinear sab."""
    cstb = np.zeros((128, _CB_NCOL), np.float32)
    for b in range(BL):
        for m in range(M):
            cstb[b * M + m, _CB_SEL32 + b] = 1.0
    cstb[0:H, _CB_ONES12] = 1.0
    cstb[0:NER, _CB_NERH : _CB_NERH + BL] = ner_slice[:, 0, :].T
    cstb[NER, _CB_NERH : _CB_NERH + BL] = 1.0
    cstb[0:NER, _CB_NERT : _CB_NERT + BL] = ner_slice[:, 1, :].T
    cstb[NER, _CB_NERT : _CB_NERT + BL] = 1.0
    # [64,128] replicators, variant v covers bl-chunk rows 16v..16v+15,
    # tiled at bases 0/64 so lhsT/rhs partition bases match
    p = np.arange(128)
    srcA = (p // 64) * 8 + (p % 64) // 8
    srcB = (p // 64) * 8 + (p % 8)
    sab64 = np.zeros((64, 8 * 128), np.float32)
    for v in range(4):
        sab64[16 * v + srcA, v * 128 + p] = 1.0
        sab64[16 * v + srcB, (4 + v) * 128 + p] = 1.0
    cstb[:, _CB_SAB:] = np.tile(sab64, (2, 1))
    return _bf16(cstb)


def _make_in_maps(inputs):
    seq = np.asarray(inputs["sequence_output"], np.float32)
    att = np.ascontiguousarray(np.asarray(inputs["attention"], np.float32))
    ner = np.asarray(inputs["ner_tags"], np.float32)
    ep = np.asarray(inputs["entity_pos"]).astype(np.int64)
    pos = ep + OFFSET  # [B, 2, M]

    whT, wtT, wbT, cstf = _weights_prep(
        np.asarray(inputs["W_head"]),
        np.asarray(inputs["b_head"]),
        np.asarray(inputs["W_tail"]),
        np.asarray(inputs["b_tail"]),
        np.asarray(inputs["W_bil"]),
        np.asarray(inputs["b_bil"]),
    )

    in_maps = []
    mh_h = np.tile(np.arange(H), M)   # gather row p = m*H + h -> h
    mh_m = np.repeat(np.arange(M), H)  # -> m
    for k in range(NCORES):
        b0 = k * BL
        seq_k = _bf16(seq[b0 : b0 + BL].reshape(BL * C, D))
        att_k = np.ascontiguousarray(att[b0 : b0 + BL].reshape(BL * H * C, C))

        idx = np.zeros((M * H, 1 + 2 * BL), np.int32)
        for b in range(BL):
            idx[b * M : (b + 1) * M, 0] = b * C + pos[b0 + b, 0, :]
            for e in range(2):
                idx[:, 1 + b * 2 + e] = (b * H + mh_h) * C + pos[b0 + b, e, mh_m]

        in_maps.append(
            {
                "seq": seq_k,
                "attn": att_k,
                "idx": idx,
                "whT": whT,
                "wtT": wtT,
                "wbT": wbT,
                "cstf": cstf,
                "cstb": _cstb_prep(ner[b0 : b0 + BL]),
            }
        )
    return in_maps


def _get_nc():
    if "nc" not in _CACHE:
        _CACHE["nc"] = _build_nc()
    return _CACHE["nc"]


def kernel(**inputs):
    global LAST_EXEC_NS, LAST_RESULTS
    nc = _get_nc()
    in_maps = _make_in_maps(inputs)
    trace = bool(int(os.environ.get("BASS_KERNEL_TRACE", "0")))
    try:
        res = run_bass_kernel_spmd(
            nc, in_maps, core_ids=list(range(NCORES)), trace=trace
        )
    except Exception:
        if not trace:
            raise
        # tracing infra unavailable in this environment -- run untraced
        res = run_bass_kernel_spmd(
            nc, in_maps, core_ids=list(range(NCORES)), trace=False
        )
    LAST_EXEC_NS = res.exec_time_ns
    LAST_RESULTS = res
    out = np.zeros((B, NCLS), np.float32)
    for k in range(NCORES):
        out[k * BL : (k + 1) * BL] = np.asarray(res.results[k]["outT"]).T
    return out



# revision 69
# speedup vs baseline: 1.3210x; 1.3210x over previous
"""Trainium2 Bass kernel for BertWithAdaThresholdLocContextPooling head.

Data-parallel over batch: 32 batches -> 8 NeuronCores x 4 batches.

v2: byte-minimized + 4-DMA-queue layout for the TRN2 cost model.
  - attention gather rows and the rs-path sequence copy are fp8 (e4m3);
    numerics verified: the context vector rs is a normalized average, so
    fp8 noise is diluted ~sqrt(512)x before it reaches the extractors.
  - extractor weights W_head/W_tail are fp8 with hs-centering: hs ~= c + d
    (c = E[logsumexp of M std normals]); the large common component c is
    routed through an exactly-precomputed f32 row-sum folded into a two-row
    bf16 bias, so fp8 quantization noise only multiplies the small residual
    d. Weights are scaled x16 into the fp8 normal range; the 1/16 is folded
    into the activations (inpT).
  - W_bil stays bf16 (fp8 there costs ~3% rel err).
  - DMAs spread over 4 queues: SP + ACT (HWDGE), DVE (HWDGE, re-enabled),
    Pool (SWDGE: both gathers + a bulk half of seq).

Math per batch b (faithful to the reference, incl. hs in BOTH extractors):
  hs  = logsumexp_m seq[pos[b,0,m]]                       [768]
  A_e = mean_m attention[:, pos[b,e,m], :]                [12, 512]
  w   = sum_h A_0 * A_1;  rs = (w @ seq[b]) / (sum(w) + 12e-5)
  x_f = tanh(W_f @ [hs | rs | ner_f | 1])   f in {head, tail}
  logits = W_bil @ vec(outer-per-group(x_head, x_tail)) + b_bil
"""

import os

import numpy as np

import concourse.bass as bass
import concourse.tile as tile
from concourse import bacc, mybir
from concourse.bass_utils import run_bass_kernel_spmd

# problem dims
B, H, C, D = 32, 12, 512, 768
M = 8
EMB, BLK = 768, 8
NCLS, NER = 97, 6
OFFSET = 1
NCORES = 8
BL = B // NCORES            # batches per core
KP = EMB * BLK              # 6144
NT = KP // 128              # 48 bilinear chunks
NJ = EMB // 128             # 6 emb chunks
CC = 2.578125               # hs centering constant (E[lse of 8 N(0,1)]), bf16-exact
SW = 16.0                   # fp8 weight scale (into e4m3 normal range)
F32 = mybir.dt.float32
BF16 = mybir.dt.bfloat16
F8 = mybir.dt.float8e4
I32 = mybir.dt.int32

# fp8 const block [128, _C8_NCOL]
_C8_SEL96 = 0               # [96,12] mention-mean selector (1/M)
_C8_SEL32 = 12              # [32,4]  mention->batch sum selector
_C8_SAB = 16                # 8 x [64,128] bilinear row replicators (tiled x2)
_C8_NCOL = 16 + 8 * 128
# bf16 const block [128, _CB_NCOL]
_CB_ONESC = 0               # [128,1]
_CB_EYE8 = 1                # [8,4]  [I4; I4] (nb-chunk rhs)
_CB_NEGC = 5                # [128,1] -CC (exp bias column)
_CB_ONE2 = 6                # [2,4] ones (bilinear-bias rhs)
_CB_ONES128 = 10            # [1,128]
_CB_NCOL = 10 + 128
KNB = 8                     # rows of the host-folded ner/bias chunk (hi/lo x 4b)

_CACHE = {}

LAST_EXEC_NS = None
LAST_RESULTS = None

# (quarter, col0, col1, out_base) pieces of each lse d-chunk j over the
# 4-way split mention rows ([128, 192] = 4 quarters x 32 (b,m) x 192 cols)
_LSE_PIECES = [
    [(0, 0, 128, 0)],
    [(0, 128, 192, 0), (1, 0, 64, 64)],
    [(1, 64, 192, 0)],
    [(2, 0, 128, 0)],
    [(2, 128, 192, 0), (3, 0, 64, 64)],
    [(3, 64, 192, 0)],
]


def _build_nc():
    nc = bacc.Bacc("TRN2", target_bir_lowering=False, debug=False)
    # hwdge = {SP, DVE} (the pre-b1a707149 config): the HW supports exactly
    # two HWDGE queues; freeing ACT to run activations un-queued is worth
    # more than its DMA slot (tables+exp+ln+tanh sit on the critical path).
    nc.hwdge_engines.discard(mybir.EngineType.Activation)
    nc.hwdge_engines.add(mybir.EngineType.DVE)
    nc.m.queues = [
        q for q in nc.m.queues if getattr(q, "name", "") != "qActDynamicHW"
    ]
    nc.m.queues.append(
        mybir.DMAQueue(
            type="dynamic",
            name="qDVEDynamicHW",
            blocks=[],
            engine=mybir.EngineType.DVE,
            location_alt=False,
            num_queues=16,
            is_HWDGE=True,
            num_semaphores=0,
            semaphores=[],
        )
    )

    seq8_h = nc.dram_tensor("seq8T", [128, 16 * D], F8, kind="ExternalInput")
    seqbs_h = nc.dram_tensor("seqbs", [BL * C * 4, D // 4], BF16, kind="ExternalInput")
    attn_h = nc.dram_tensor("attn8", [BL * H * C, C], F8, kind="ExternalInput")
    idx_h = nc.dram_tensor("idx", [128, 9], I32, kind="ExternalInput")
    wh8_h = nc.dram_tensor("wh8", [128, 12 * EMB], F8, kind="ExternalInput")
    wt8_h = nc.dram_tensor("wt8", [128, 12 * EMB], F8, kind="ExternalInput")
    # nbT[p, half, j, m]: rows 0..3 = hi(batch p), 4..7 = lo; the host folds
    # W_ner @ ner + bias + CC*rowsum(W_hs) into this per-batch constant.
    # cols [2*NJ*128 : +NCLS] rows 0..1 hold the b_bil (hi, lo) pair.
    nbt_h = nc.dram_tensor("nbT", [KNB, 2 * NJ * 128 + NCLS], BF16, kind="ExternalInput")
    wb_h = nc.dram_tensor("wbT", [128, NT * NCLS], BF16, kind="ExternalInput")
    cst8_h = nc.dram_tensor("cst8", [128, _C8_NCOL], F8, kind="ExternalInput")
    cstb_h = nc.dram_tensor("cstb", [128, _CB_NCOL], BF16, kind="ExternalInput")
    out_h = nc.dram_tensor("outT", [NCLS, BL], F32, kind="ExternalOutput")

    AF = mybir.ActivationFunctionType
    OP = mybir.AluOpType

    with tile.TileContext(nc) as tc:
        with (
            tc.tile_pool(name="w", bufs=1) as wp,
            tc.tile_pool(name="seqp", bufs=1) as sp,
            tc.tile_pool(name="g", bufs=2) as gp,
            tc.tile_pool(name="ps", bufs=8, space="PSUM") as pp,
        ):
            # ---- SP queue: idx (gates gathers), nbT, wh8, wt8-rs, wb piece
            idx_sb = wp.tile([128, 9], I32)
            nc.sync.dma_start(out=idx_sb[:], in_=idx_h[:])
            nbt_sb = wp.tile([KNB, 2 * NJ * 128 + NCLS], BF16)
            nc.sync.dma_start(out=nbt_sb[:], in_=nbt_h[:])
            wh8_sb = wp.tile([128, 12 * EMB], F8)
            nc.sync.dma_start(out=wh8_sb[:], in_=wh8_h[:])
            wt8_sb = wp.tile([128, 12 * EMB], F8)
            nc.sync.dma_start(out=wt8_sb[:, 6 * EMB :], in_=wt8_h[:, 6 * EMB :])
            wb_sb = wp.tile([128, NT * NCLS], BF16)
            nc.sync.dma_start(out=wb_sb[:, 32 * NCLS :], in_=wb_h[:, 32 * NCLS :])

            # ---- ACT queue: compute-only. Preload the exp+ln table (set 6)
            # so the table-load pass doesn't first-fit Exp into the tanh set
            ld6 = mybir.InstLoadActFuncSet(act_func_set_id=6)
            ld6.engine = mybir.EngineType.Activation
            nc.scalar.add_instruction(ld6)

            # ---- DVE queue: the whole rs-path sequence in one DMA
            sq = sp.tile([128, 16, D], F8, name="sq")
            nc.vector.dma_start(
                out=sq[:, :, :],
                in_=seq8_h[:, :].rearrange("p (t d) -> p t d", d=D),
            )

            def seqtile(t):
                return sq[:, t, :]

            # ---- Pool queue: consts fill the idx-wait, gathers, wt8-hs, wb
            cstb_sb = wp.tile([128, _CB_NCOL], BF16)
            nc.gpsimd.dma_start(out=cstb_sb[:], in_=cstb_h[:])
            cst8_sb = wp.tile([128, _C8_NCOL], F8)
            nc.gpsimd.dma_start(out=cst8_sb[:], in_=cst8_h[:])

            sel96 = cst8_sb[0 : M * H, _C8_SEL96 : _C8_SEL96 + H]
            onescol = cstb_sb[0:128, _CB_ONESC : _CB_ONESC + 1]
            ones128 = cstb_sb[0:1, _CB_ONES128 : _CB_ONES128 + 128]
            eye8 = cstb_sb[0:KNB, _CB_EYE8 : _CB_EYE8 + BL]
            negc = cstb_sb[0:128, _CB_NEGC : _CB_NEGC + 1]

            # wb pieces fill the remaining idx-wait gap in small steps so
            # the greedy per-queue scheduler cannot slide a long DMA in
            # front of the att gather the moment before idx lands
            nc.gpsimd.dma_start(
                out=wb_sb[:, : 8 * NCLS], in_=wb_h[:, : 8 * NCLS]
            )
            nc.gpsimd.dma_start(
                out=wb_sb[:, 8 * NCLS : 14 * NCLS],
                in_=wb_h[:, 8 * NCLS : 14 * NCLS],
            )
            nc.gpsimd.dma_start(
                out=wb_sb[:, 14 * NCLS : 16 * NCLS],
                in_=wb_h[:, 14 * NCLS : 16 * NCLS],
            )
            # gathers (att first: longest downstream chain). One gather per
            # (b,e) slot with a column idx AP: the multi-column batched form
            # returns garbage on real HW (sim-only semantics).
            att_g = gp.tile([M * H, 2 * BL, C], F8, name="att_g")
            for s in range(2 * BL):
                nc.gpsimd.indirect_dma_start(
                    out=att_g[:, s, :],
                    out_offset=None,
                    in_=attn_h[:],
                    in_offset=bass.IndirectOffsetOnAxis(
                        ap=idx_sb[0 : M * H, s : s + 1], axis=0
                    ),
                )
            g_ment = gp.tile([128, D // 4], BF16, name="g_ment")
            nc.gpsimd.indirect_dma_start(
                out=g_ment[:, :],
                out_offset=None,
                in_=seqbs_h[:],
                in_offset=bass.IndirectOffsetOnAxis(ap=idx_sb[:, 8:9], axis=0),
            )
            # the tail hs-half weight, then the mid wb piece
            nc.gpsimd.dma_start(
                out=wt8_sb[:, : 6 * EMB], in_=wt8_h[:, : 6 * EMB]
            )
            nc.gpsimd.dma_start(
                out=wb_sb[:, 16 * NCLS : 32 * NCLS],
                in_=wb_h[:, 16 * NCLS : 32 * NCLS],
            )

            scr = wp.tile([1, 2], BF16)

            # ---- phase 1 (ACT part): mention exp, centered by CC via bias
            expm = gp.tile([128, D // 4], BF16, name="expm")
            nc.scalar.activation(expm[:, :], g_ment[:, :], AF.Exp, bias=negc)

            # ---- phase 2: attention means -> raw context weights wT_f
            # PT matmuls are issued BEFORE the lse matmuls: the PE queue is
            # in-order, and the attention chain must not stall on exp.
            PT = pp.tile([128, BL, 4, 2, H], F32, tag="ps", name="PT")
            for b in range(BL):
                for c in range(4):
                    for e in range(2):
                        nc.tensor.matmul(
                            out=PT[:, b, c, e, :],
                            lhsT=att_g[:, b * 2 + e, c * 128 : (c + 1) * 128],
                            rhs=sel96,
                            start=True,
                            stop=True,
                        )

            # ---- phase 1 (PE part): logsumexp partial sums
            lse_ps = pp.tile([128, NJ, BL], F32, tag="ps", name="lse_ps")
            for j in range(NJ):
                for q, c0, c1, ob in _LSE_PIECES[j]:
                    nc.tensor.matmul(
                        out=lse_ps[ob : ob + (c1 - c0), j, :],
                        lhsT=expm[q * 32 : (q + 1) * 32, c0:c1],
                        rhs=cst8_sb[
                            q * 32 : (q + 1) * 32, _C8_SEL32 : _C8_SEL32 + BL
                        ],
                        start=True,
                        stop=True,
                        tile_position=(q * 32, ob),
                    )
            lse_ln = wp.tile([128, NJ, BL], F32)
            nc.scalar.activation(lse_ln[:, :, :], lse_ps[:, :, :], AF.Ln)
            # Tanh-table prefetch pinned after Ln (reads Ln's output so the
            # scheduler cannot float it ahead of the Exp/Ln table window)
            nc.scalar.activation(scr[0:1, 1:2], lse_ln[0:1, 0:1, 0:1], AF.Tanh)

            # inpT[:, c, :] = chunk c of [hs-CC | rs*srec] / 16, bf16
            # (the hs-mul is issued later, after the phase-2 DVE chain, so it
            # does not block the attention-critical DVE FIFO)
            inpT = wp.tile([128, 12, BL], BF16)
            pte0 = gp.tile([128, BL, 4, H], F32, name="pte0")
            nc.vector.tensor_copy(pte0[:, :, :, :], PT[:, :, :, 0, :])
            prodT = gp.tile([128, BL, 4, H], F32, name="prodT")
            nc.vector.tensor_tensor(
                out=prodT[:, :, :, :],
                in0=pte0[:, :, :, :],
                in1=PT[:, :, :, 1, :],
                op=OP.mult,
            )
            wT_f = wp.tile([128, BL, 4], BF16)
            with nc.allow_low_precision(reason="12-way head sum rounds to bf16"):
                nc.vector.reduce_sum(
                    out=wT_f[:, :, :], in_=prodT[:, :, :, :], axis=mybir.AxisListType.X
                )
            # s_b = sum(w) on PE, then 1/(16*(s+H*1e-5)) broadcast to sb16
            s_ps = pp.tile([1, BL, 4], F32, tag="ps", name="s_ps")
            nc.tensor.matmul(
                out=s_ps[:, :, :],
                lhsT=onescol,
                rhs=wT_f[:, :, :].rearrange("p b c -> p (b c)"),
                start=True,
                stop=True,
            )
            nc.vector.tensor_scalar_mul(
                out=inpT[:, 0:NJ, :], in0=lse_ln[:, :, :], scalar1=1.0 / SW
            )
            sraw = wp.tile([1, BL], F32)
            nc.vector.reduce_sum(
                out=sraw[:], in_=s_ps[:, :, :], axis=mybir.AxisListType.X
            )
            sden = wp.tile([1, BL], F32)
            nc.vector.tensor_scalar_add(
                out=sden[:], in0=sraw[:], scalar1=float(H) * 1e-5
            )
            srec = wp.tile([1, BL], BF16)
            with nc.allow_low_precision(reason="normalizer rounds to bf16"):
                nc.vector.reciprocal(out=srec[:], in_=sden[:])
            sb_ps = pp.tile([128, BL], F32, tag="ps", name="sb_ps")
            nc.tensor.matmul(
                out=sb_ps[:], lhsT=ones128, rhs=srec[:], start=True, stop=True
            )
            sb16 = wp.tile([128, 1, BL], BF16)
            nc.vector.tensor_scalar_mul(
                out=sb16[:, 0, :], in0=sb_ps[:], scalar1=1.0 / SW
            )

            # ---- phase 3: rs_raw = w_raw @ seq (fp8 seq tiles, raw weights)
            rsT = [
                pp.tile([128, 2, BL], F32, tag="ps", name=f"rsT{k}") for k in range(3)
            ]
            for b in range(BL):
                for j in range(NJ):
                    for c in range(4):
                        nc.tensor.matmul(
                            out=rsT[j % 3][:, j // 3, b : b + 1],
                            lhsT=seqtile(b * 4 + c)[:, j * 128 : (j + 1) * 128],
                            rhs=wT_f[:, b, c : c + 1],
                            start=(c == 0),
                            stop=(c == 3),
                        )
            # inpT rs chunks = rsT * (srec/16) broadcast over the j dim
            for k in range(3):
                nc.vector.tensor_tensor(
                    out=inpT[:, 6 + k : 12 : 3, :],
                    in0=rsT[k][:, :, :],
                    in1=sb16[:, :, :].to_broadcast([128, 2, BL]),
                    op=OP.mult,
                )
            # ---- phase 4: extractors (fp8 weights, bf16 ner/bias chunk)
            # rs chunks first, hs last: the tail hs-half weight (Pool) is the
            # latest-arriving extractor operand
            ex_ps = pp.tile([128, 2, NJ, BL], F32, tag="ps", name="ex_ps")
            corders = (
                list(range(6, 12)) + [12] + list(range(6)),   # head: one sem
                list(range(6)) + [12] + list(range(6, 12)),   # tail: hs first
            )
            for half, w8 in enumerate((wh8_sb, wt8_sb)):
                for j in range(NJ):
                    for ci, c in enumerate(corders[half]):
                        if c < 12:
                            l = w8[:, c * EMB + j * 128 : c * EMB + (j + 1) * 128]
                            r = inpT[:, c, :]
                        else:
                            l = nbt_sb[0:KNB, (half * NJ + j) * 128 : (half * NJ + j + 1) * 128]
                            r = eye8
                        nc.tensor.matmul(
                            out=ex_ps[:, half, j, :],
                            lhsT=l,
                            rhs=r,
                            start=(ci == 0),
                            stop=(ci == 12),
                        )
            ex_fT = wp.tile([128, 2, NJ, BL], BF16)
            nc.scalar.activation(ex_fT[:, :, :, :], ex_ps[:, :, :, :], AF.Tanh)

            # ---- phase 5: grouped bilinear + output matmul
            # one matmul per (side, r): all 6 j-chunks ride as 24 rhs columns
            psA = pp.tile([128, 8, NJ, BL], F32, tag="ps", name="psA")
            psB = pp.tile([128, 8, NJ, BL], F32, tag="ps", name="psB")
            for r in range(8):
                base, v = 64 * (r // 4), r % 4
                selA = cst8_sb[base : base + 64, _C8_SAB + v * 128 : _C8_SAB + (v + 1) * 128]
                selB = cst8_sb[base : base + 64,
                               _C8_SAB + (4 + v) * 128 : _C8_SAB + (5 + v) * 128]
                nc.tensor.matmul(
                    out=psA[:, r, :, :].rearrange("p j b -> p (j b)"), lhsT=selA,
                    rhs=ex_fT[base : base + 64, 0, :, :].rearrange("p j b -> p (j b)"),
                    start=True, stop=True,
                )
                nc.tensor.matmul(
                    out=psB[:, r, :, :].rearrange("p j b -> p (j b)"), lhsT=selB,
                    rhs=ex_fT[base : base + 64, 1, :, :].rearrange("p j b -> p (j b)"),
                    start=True, stop=True,
                )
            # halves pipeline: blT of j 0..2 feeds logits while j 3..5 multiply
            pteA = gp.tile([128, 8, NJ, BL], F32, name="pteA")
            blT = gp.tile([128, 8, NJ, BL], BF16, name="blT")
            logit_ps = pp.tile([NCLS, BL], F32, tag="ps", name="logit_ps")
            # b_bil enters as chunk -1: two bf16 rows (hi/lo) x ones rhs
            nc.tensor.matmul(
                out=logit_ps[:],
                lhsT=nbt_sb[0:2, 2 * NJ * 128 : 2 * NJ * 128 + NCLS],
                rhs=cstb_sb[0:2, _CB_ONE2 : _CB_ONE2 + BL],
                start=True,
                stop=False,
            )
            nc.vector.tensor_copy(pteA[:, :, :, :], psA[:, :, :, :])
            nc.vector.tensor_tensor(
                out=blT[:, :, :, :],
                in0=pteA[:, :, :, :],
                in1=psB[:, :, :, :],
                op=OP.mult,
            )
            for t in range(NT):
                nc.tensor.matmul(
                    out=logit_ps[:],
                    lhsT=wb_sb[:, t * NCLS : (t + 1) * NCLS],
                    rhs=blT[:, t % 8, t // 8, :],
                    start=False,
                    stop=(t == NT - 1),
                )
            logitsT_sb = wp.tile([NCLS, BL], F32)
            nc.vector.tensor_copy(logitsT_sb[:], logit_ps[:])
            nc.sync.dma_start(out=out_h[:], in_=logitsT_sb[:])

    nc.compile()
    return nc


def _bf16(x):
    import ml_dtypes

    return np.ascontiguousarray(np.asarray(x).astype(ml_dtypes.bfloat16))


def _f8(x):
    import ml_dtypes

    return np.ascontiguousarray(np.asarray(x).astype(ml_dtypes.float8_e4m3))


def _weights_prep(W_head, b_head, W_tail, b_tail, W_bil, b_bil):
    """Host-side packing: fp8 x16 transposed main weights, bf16 ner/bias
    chunk with the hs-centering row-sum folded in (two-row bf16 split)."""
    import ml_dtypes

    def bf16_pair(v):
        hi = v.astype(ml_dtypes.bfloat16).astype(np.float32)
        return hi, v - hi

    def pack(Wf, bf):
        Wf = np.asarray(Wf, np.float32)
        w8 = np.zeros((128, 12 * EMB), np.float32)
        wt = Wf.T  # [2D+NER, EMB]
        for c in range(12):
            w8[:, c * EMB : (c + 1) * EMB] = SW * wt[c * 128 : (c + 1) * 128, :]
        # per-extractor constants for the host-folded nb chunk:
        # corr (bias + centering rowsum) and the ner columns
        corr = np.asarray(bf, np.float32) + CC * Wf[:, :D].sum(axis=1)
        return _f8(w8), corr, Wf[:, 2 * D :].astype(np.float32)

    wh8, corr_h, wner_h = pack(W_head, b_head)
    wt8, corr_t, wner_t = pack(W_tail, b_tail)
    bbil_pair = bf16_pair(np.asarray(b_bil, np.float32))

    wbe = np.asarray(W_bil, np.float32).T  # [KP, NCLS]
    wbT = _bf16(wbe.reshape(NT, 128, NCLS).transpose(1, 0, 2).reshape(128, NT * NCLS))

    cst8 = np.zeros((128, _C8_NCOL), np.float32)
    for m in range(M):
        for h in range(H):
            cst8[m * H + h, _C8_SEL96 + h] = 1.0 / M
    for q in range(4):
        for b in range(BL):
            for m in range(M):
                cst8[q * 32 + b * M + m, _C8_SEL32 + b] = 1.0
    p = np.arange(128)
    srcA = (p // 64) * 8 + (p % 64) // 8
    srcB = (p // 64) * 8 + (p % 8)
    sab64 = np.zeros((64, 8 * 128), np.float32)
    for v in range(4):
        sab64[16 * v + srcA, v * 128 + p] = 1.0
        sab64[16 * v + srcB, (4 + v) * 128 + p] = 1.0
    cst8[:, _C8_SAB:] = np.tile(sab64, (2, 1))
    return wh8, wt8, wbT, _f8(cst8), (corr_h, wner_h, corr_t, wner_t, bbil_pair)


def _cstb_prep():
    cstb = np.zeros((128, _CB_NCOL), np.float32)
    cstb[0:128, _CB_ONESC] = 1.0
    for b in range(BL):
        cstb[b, _CB_EYE8 + b] = 1.0
        cstb[BL + b, _CB_EYE8 + b] = 1.0
    cstb[0:128, _CB_NEGC] = -CC
    cstb[0:2, _CB_ONE2 : _CB_ONE2 + BL] = 1.0
    cstb[0:1, _CB_ONES128 : _CB_ONES128 + 128] = 1.0
    return _bf16(cstb)


def _nbt_prep(ner_slice, consts):
    """Per-core [8, 2*NJ*128 + NCLS] bf16 hi/lo pair of the folded ner+bias
    chunk: nb[half][b] = corr_half + W_ner_half @ ner[b, half]."""
    import ml_dtypes

    corr_h, wner_h, corr_t, wner_t, bbil_pair = consts
    nbt = np.zeros((KNB, 2 * NJ * 128 + NCLS), np.float32)
    for half, (corr, wner) in enumerate(((corr_h, wner_h), (corr_t, wner_t))):
        for b in range(BL):
            nb = corr + wner @ ner_slice[b, half]  # [EMB] f32
            hi = nb.astype(ml_dtypes.bfloat16).astype(np.float32)
            nbt[b, half * NJ * 128 : (half + 1) * NJ * 128] = hi
            nbt[BL + b, half * NJ * 128 : (half + 1) * NJ * 128] = nb - hi
    nbt[0, 2 * NJ * 128 :] = bbil_pair[0]
    nbt[1, 2 * NJ * 128 :] = bbil_pair[1]
    return _bf16(nbt)


def _make_in_maps(inputs):
    seq = np.asarray(inputs["sequence_output"], np.float32)
    att = np.asarray(inputs["attention"], np.float32)
    ner = np.asarray(inputs["ner_tags"], np.float32)
    ep = np.asarray(inputs["entity_pos"]).astype(np.int64)
    pos = ep + OFFSET  # [B, 2, M]

    wh8, wt8, wbT, cst8, nbconsts = _weights_prep(
        np.asarray(inputs["W_head"]),
        np.asarray(inputs["b_head"]),
        np.asarray(inputs["W_tail"]),
        np.asarray(inputs["b_tail"]),
        np.asarray(inputs["W_bil"]),
        np.asarray(inputs["b_bil"]),
    )

    in_maps = []
    mh_h = np.tile(np.arange(H), M)    # gather row p = m*H + h -> h
    mh_m = np.repeat(np.arange(M), H)  # -> m
    for k in range(NCORES):
        b0 = k * BL
        seq_k = seq[b0 : b0 + BL].reshape(BL * C, D)
        seq8T = np.zeros((128, 16 * D), np.float32)
        for t in range(16):
            seq8T[:, t * D : (t + 1) * D] = seq_k[t * 128 : (t + 1) * 128, :]
        seqbs = _bf16(seq_k.reshape(BL * C * 4, D // 4))
        att_k = _f8(att[b0 : b0 + BL].reshape(BL * H * C, C))

        idx = np.zeros((128, 9), np.int32)
        for b in range(BL):
            for e in range(2):
                idx[0 : M * H, b * 2 + e] = (b * H + mh_h) * C + pos[b0 + b, e, mh_m]
        for q in range(4):
            for b in range(BL):
                for m in range(M):
                    idx[q * 32 + b * M + m, 8] = (b * C + pos[b0 + b, 0, m]) * 4 + q

        in_maps.append(
            {
                "seq8T": _f8(seq8T),
                "seqbs": seqbs,
                "attn8": att_k,
                "idx": idx,
                "wh8": wh8,
                "nbT": _nbt_prep(ner[b0 : b0 + BL], nbconsts),
                "wt8": wt8,
                "wbT": wbT,
                "cst8": cst8,
                "cstb": _cstb_prep(),
            }
        )
    return in_maps


def _get_nc():
    if "nc" not in _CACHE:
        _CACHE["nc"] = _build_nc()
    return _CACHE["nc"]


def kernel(**inputs):
    global LAST_EXEC_NS, LAST_RESULTS
    nc = _get_nc()
    in_maps = _make_in_maps(inputs)
    trace = bool(int(os.environ.get("BASS_KERNEL_TRACE", "0")))
    try:
        res = run_bass_kernel_spmd(
            nc, in_maps, core_ids=list(range(NCORES)), trace=trace
        )
    except Exception:
        if not trace:
            raise
        # tracing infra unavailable in this environment -- run untraced
        res = run_bass_kernel_spmd(
            nc, in_maps, core_ids=list(range(NCORES)), trace=False
        )
    LAST_EXEC_NS = res.exec_time_ns
    LAST_RESULTS = res
    out = np.zeros((B, NCLS), np.float32)
    for k in range(NCORES):
        out[k * BL : (k + 1) * BL] = np.asarray(res.results[k]["outT"]).T
    return out
